# revision 38
# baseline (speedup 1.0000x reference)
"""Trainium2 Bass kernel for multi-head attention (B=4, S=1024, D=1024, H=16).

Sharding: 8 cores = batch(4) x query-half(2). Each core computes the full
attention output for its 512 query rows of its batch (all 16 heads), so the
per-core outputs are disjoint slices of the final [4, 1024, 1024] output and
the gather is a pure concatenation -- no cross-core communication.

Host-side prep (part of the sharding step): inputs are cast to bf16 and
pre-transposed so every DRAM tensor is already in the layout the matmuls
consume (x^T with d_model on partitions, weights in natural [c, hd] layout).
This removes all on-device casts/transposes and halves HBM load traffic.

Per-core dataflow (all matmuls bf16, fp32 PSUM accumulation):
  qT = Wq^T xqT + bq   kT = Wk^T xkT + bk   (bias fused into the PSUM
    drain as a DVE per-partition scalar add)
  v[sk, h, d|1] = xvT^T Wv + bv             (ones column -> rowsum; bias
    fused into the drain via a partition-replicated bias tile)
  S^T[h] per head pair: even head on PE rows 0-63, odd on 64-127
    (row-packed tile_position -> the two streams run concurrently)
  P^T = exp(S^T / 8)          (ScalarE, PSUM -> bf16 SBUF)
  [out^T[h]; rowsum] = [v_h | 1]^T P^T
  attn^T = out^T * (1/rowsum) (wide [128,8] reciprocal via DRAM bounce +
    stride-0 partition-broadcast DMA)
  o = attn^T^T Wo + bo

Scheduling: loads are dependency-chained in consumption order across the
two HWDGE rings (x on sync, weights on scalar) so the first-needed tensors
get full HBM bandwidth; emission interleaves v/k/q projections with
attention head pairs so the tensor engine always has dense work while
ScalarE chews through the exps (keeps the PE HAM clock-gate warm); per-jt
output tiles keep consumer dependencies exact so the output projection
overlaps the last heads' normalize chains.
"""

import sys

if "/opt/trn_rl_repo" not in sys.path:
    sys.path.insert(0, "/opt/trn_rl_repo")

import numpy as np

B = 4
S = 1024
C = 1024          # d_model
H = 16            # heads
D = 64            # head dim
HD = H * D        # 1024
SQ = S // 2       # queries per core
NCORES = 8
SCALE = 0.125     # 1/sqrt(D)

CT = C // 128     # 8 contraction tiles
JT = HD // 128    # 8 head-pair tiles
SKT = S // 128    # 8 key tiles
STQ = SQ // 128   # 4 query row-tiles

_CACHED = {}


def _emit(tc, ctx):
    import concourse.bass as bass
    from concourse import mybir

    nc = tc.nc
    f32 = mybir.dt.float32
    bf16 = mybir.dt.bfloat16
    Exp = mybir.ActivationFunctionType.Exp

    # ---- DRAM I/O (bf16, pre-transposed AND partition-major packed on
    # host: [128, ct, m] so every partition line is one contiguous run) ----
    xqt = nc.dram_tensor("xqt", [128, CT, SQ], bf16, kind="ExternalInput").ap()
    xkt = nc.dram_tensor("xkt", [128, CT, S], bf16, kind="ExternalInput").ap()
    xvt = nc.dram_tensor("xvt", [128, SKT, CT, 128], bf16, kind="ExternalInput").ap()
    wq = nc.dram_tensor("wq", [128, CT, HD], bf16, kind="ExternalInput").ap()
    wk = nc.dram_tensor("wk", [128, CT, HD], bf16, kind="ExternalInput").ap()
    wv = nc.dram_tensor("wv", [128, 2, CT, 512], bf16, kind="ExternalInput").ap()
    wo = nc.dram_tensor("wo", [128, JT, C], bf16, kind="ExternalInput").ap()
    bq = nc.dram_tensor("bq", [HD], f32, kind="ExternalInput").ap()
    bk = nc.dram_tensor("bk", [HD], f32, kind="ExternalInput").ap()
    bv = nc.dram_tensor("bv", [HD], f32, kind="ExternalInput").ap()
    bo = nc.dram_tensor("bo", [C], f32, kind="ExternalInput").ap()
    out = nc.dram_tensor("out", [SQ, C], f32, kind="ExternalOutput").ap()

    # rowsum bounce rows (reshape for wide reciprocal + partition broadcast)
    rs_scr = nc.dram_tensor("rs_scr", [H, SQ], f32).ap()
    rr_scr = nc.dram_tensor("rr_scr", [H, SQ], f32).ap()

    # ---- long-lived SBUF ----
    persist = ctx.enter_context(tc.tile_pool(name="persist", bufs=1))
    wq_sb = persist.tile([128, CT, HD], bf16)
    wk_sb = persist.tile([128, CT, HD], bf16)
    wv_sb = persist.tile([128, 2, CT, 512], bf16)
    wo_sb = persist.tile([128, JT, C], bf16)
    xqT = persist.tile([128, CT, SQ], bf16)
    xkT = persist.tile([128, CT, S], bf16)
    xvT = persist.tile([128, SKT, CT, 128], bf16)
    # per-jt / per-half tiles so consumers depend only on their producers
    qT = [persist.tile([128, SQ], bf16, name=f"qT{j}") for j in range(JT)]
    kT = [persist.tile([128, S], bf16, name=f"kT{j}") for j in range(JT)]
    v_sb = [
        persist.tile([128, SKT, H // 2, D + 1], bf16, name=f"vsb{b}")
        for b in range(2)
    ]
    aoT = [persist.tile([128, SQ], bf16, name=f"aoT{j}") for j in range(JT)]
    bq_col = persist.tile([128, JT], f32)
    bk_col = persist.tile([128, JT], f32)
    bv_bc = persist.tile([128, HD], f32)
    bo_bc = persist.tile([128, C], f32)
    ones65 = persist.tile([65, 64], f32)

    # ---- working pools ----
    pj = ctx.enter_context(tc.tile_pool(name="proj_psum", bufs=2, space="PSUM"))
    sp = ctx.enter_context(tc.tile_pool(name="st_psum", bufs=2, space="PSUM"))
    vp = ctx.enter_context(tc.tile_pool(name="pv_psum", bufs=2, space="PSUM"))
    pt_pool = ctx.enter_context(tc.tile_pool(name="pt", bufs=12))
    of_pool = ctx.enter_context(tc.tile_pool(name="of", bufs=4))
    rsp_pool = ctx.enter_context(tc.tile_pool(name="rsp", bufs=2))
    rb_pool = ctx.enter_context(tc.tile_pool(name="rb", bufs=3))
    ao_pool = ctx.enter_context(tc.tile_pool(name="ao_stage", bufs=3))
    out_pool = ctx.enter_context(tc.tile_pool(name="out_sb", bufs=3))

    # ---- loads: nc.sync HWDGE ring is FIFO per engine, so emission order
    # is arrival order. vproj work comes first in the schedule, so xv/wv
    # lead; remaining tensors follow in consumption order, split so the
    # first consumer can start at partial load. Tiny bias rows ride the
    # gpsimd (SWDGE) ring so they don't delay the big streams.
    # biases: bq/bk as per-partition columns (fused into the PSUM-drain
    # copies), bv/bo replicated across partitions via stride-0 DMA.
    with nc.allow_non_contiguous_dma(reason="tiny transposed bias loads"):
        nc.gpsimd.dma_start(out=bq_col[:, :], in_=bq.rearrange("(t p) -> p t", p=128))
        nc.gpsimd.dma_start(out=bk_col[:, :], in_=bk.rearrange("(t p) -> p t", p=128))
    for dst, src in ((bv_bc, bv), (bo_bc, bo)):
        s = src.rearrange("(o m) -> o m", o=1)
        rep = bass.AP(tensor=s.tensor, offset=s.offset, ap=[[0, 128]] + s.ap[1:])
        nc.gpsimd.dma_start(out=dst[:, :], in_=rep)
    # x tensors on the sync HWDGE ring, weights on the scalar HWDGE ring.
    # Transfers are grouped in consumption-order pairs; each group is
    # dependency-chained on the previous so two transfers at a time split
    # the HBM bandwidth instead of all of them at once.
    from concourse.bass import _add_dep_helper

    d_xv0 = nc.sync.dma_start(out=xvT[:, 0:4, :, :], in_=xvt[:, 0:4, :, :])
    d_wv0 = nc.scalar.dma_start(out=wv_sb[:, 0, :, :], in_=wv[:, 0, :, :])
    d_xv1 = nc.sync.dma_start(out=xvT[:, 4:8, :, :], in_=xvt[:, 4:8, :, :])
    d_wv1 = nc.scalar.dma_start(out=wv_sb[:, 1, :, :], in_=wv[:, 1, :, :])
    d_xk = nc.sync.dma_start(out=xkT[:, :, :], in_=xkt[:, :, :])
    d_wk = nc.scalar.dma_start(out=wk_sb[:, :, :], in_=wk[:, :, :])
    d_xq = nc.sync.dma_start(out=xqT[:, :, :], in_=xqt[:, :, :])
    d_wq = nc.scalar.dma_start(out=wq_sb[:, :, :], in_=wq[:, :, :])
    d_wo = nc.scalar.dma_start(out=wo_sb[:, :, :], in_=wo[:, :, :])
    for later, earlier in (
        (d_xv1, d_xv0),
        (d_wv1, d_wv0),
        (d_xk, d_xv1),
        (d_wk, d_wv1),
        (d_xq, d_xk),
        (d_wq, d_wk),
        (d_wo, d_wq),
    ):
        _add_dep_helper(later.ins, earlier.ins, sync=True, reason="load order")

    nc.vector.memset(ones65[:, :], 1.0)
    nc.vector.memset(v_sb[0][:, :, :, D : D + 1], 1.0)
    nc.vector.memset(v_sb[1][:, :, :, D : D + 1], 1.0)

    def kqproj(w_sb, x_sb, b_col, o_sb, jt, nsb):
        # o^T[j, s] = W^T x^T + b  (contraction over c; bias fused into the
        # PSUM drain as a per-partition scalar add)
        for sb in range(nsb):
            ps = pj.tile([128, 512], f32, tag="pp")
            for ct in range(CT):
                nc.tensor.matmul(
                    ps[:, :],
                    lhsT=w_sb[:, ct, jt * 128 : (jt + 1) * 128],
                    rhs=x_sb[:, ct, sb * 512 : (sb + 1) * 512],
                    start=(ct == 0),
                    stop=(ct == CT - 1),
                )
            nc.vector.tensor_scalar_add(
                out=o_sb[jt][:, sb * 512 : (sb + 1) * 512],
                in0=ps[:, :],
                scalar1=b_col[:, jt : jt + 1],
            )

    def vproj(skt, hb):
        # v[sk, hd-half] = xv^T^T Wv + bv (bias fused into the PSUM drain)
        ps = pj.tile([128, 512], f32, tag="pp")
        for ct in range(CT):
            nc.tensor.matmul(
                ps[:, :],
                lhsT=xvT[:, skt, ct, :],
                rhs=wv_sb[:, hb, ct, :],
                start=(ct == 0),
                stop=(ct == CT - 1),
            )
        nc.vector.tensor_add(
            out=v_sb[hb][:, skt, :, 0:D],
            in0=ps.rearrange("p (h d) -> p h d", d=D),
            in1=bv_bc[:, hb * 512 : (hb + 1) * 512].rearrange(
                "p (h d) -> p h d", d=D
            ),
        )

    pt_live = {}

    def scores_pair(jt):
        # even head on PE rows 0-63, odd head on rows 64-127 (concurrent)
        pe, po = [], []
        for g in range(4):
            st_e = sp.tile([128, 2, 512], f32, tag="st")
            st_o = sp.tile([128, 2, 512], f32, tag="st")
            for i in range(2):
                skt = 2 * g + i
                nc.tensor.matmul(
                    st_e[:, i, :],
                    lhsT=kT[jt][0:64, skt * 128 : (skt + 1) * 128],
                    rhs=qT[jt][0:64, :],
                    start=True,
                    stop=True,
                )
                nc.tensor.matmul(
                    st_o[:, i, :],
                    lhsT=kT[jt][64:128, skt * 128 : (skt + 1) * 128],
                    rhs=qT[jt][64:128, :],
                    start=True,
                    stop=True,
                )
            p_e = pt_pool.tile([128, 2, 512], bf16, tag="pt")
            p_o = pt_pool.tile([128, 2, 512], bf16, tag="pt")
            nc.scalar.activation(out=p_e[:, :, :], in_=st_e[:, :, :], func=Exp, scale=SCALE)
            nc.scalar.activation(out=p_o[:, :, :], in_=st_o[:, :, :], func=Exp, scale=SCALE)
            pe.append(p_e)
            po.append(p_o)
        pt_live[2 * jt] = pe
        pt_live[2 * jt + 1] = po

    of_live = {}

    def pv_head(h):
        pts = pt_live.pop(h)
        o_ps = vp.tile([65, 512], f32, tag="pv")
        for skt in range(SKT):
            nc.tensor.matmul(
                o_ps[:, :],
                lhsT=v_sb[h // 8][:, skt, h % 8, :],
                rhs=pts[skt // 2][:, skt % 2, :],
                start=(skt == 0),
                stop=(skt == SKT - 1),
            )
        # one copy frees the PSUM bank immediately (rows 0-63 out^T, 64 sum)
        o_f = of_pool.tile([65, 512], f32, tag="of")
        nc.vector.tensor_copy(out=o_f[:, :], in_=o_ps[:, :])
        nc.gpsimd.dma_start(out=rs_scr[h : h + 1, :], in_=o_f[64:65, :])
        of_live[h] = o_f

    def pair_norm(jt):
        he, ho = 2 * jt, 2 * jt + 1
        if jt == JT - 1:
            # last pair: its chain is exposed at the tail, so trade a slow
            # [1,512] DVE reciprocal for a DRAM-free path (broadcast via a
            # K=1 matmul into a spare score-psum slot).
            for h in (he, ho):
                o_f = of_live.pop(h)
                rsr = of_pool.tile([65, 512], f32, tag="of")
                nc.vector.reciprocal(out=rsr[64:65, :], in_=o_f[64:65, :])
                rbp = sp.tile([128, 2, 512], f32, tag="st")
                nc.tensor.matmul(
                    rbp[0:64, 0, :],
                    lhsT=ones65[64:65, 0:64],
                    rhs=rsr[64:65, :],
                    start=True,
                    stop=True,
                )
                if h % 2 == 0:
                    nc.vector.tensor_mul(
                        out=aoT[jt][0:64, :], in0=o_f[0:64, :], in1=rbp[0:64, 0, :]
                    )
                else:
                    ao_s = ao_pool.tile([64, 512], bf16, tag="aos")
                    nc.vector.tensor_mul(
                        out=ao_s[:, :], in0=o_f[0:64, :], in1=rbp[0:64, 0, :]
                    )
                    nc.scalar.dma_start(out=aoT[jt][64:128, :], in_=ao_s[:, :])
            return
        # reciprocal done wide ([128,8] for the head pair via DRAM bounce:
        # DVE recip cost is per-lane free-size, so [1,512] is ~9x slower).
        rsp = rsp_pool.tile([128, 2, 2, 4], f32, tag="rsp")
        nc.gpsimd.dma_start(
            out=rsp[:, 0, :, :],
            in_=rs_scr[he : he + 2, :].rearrange("h (p q) -> p h q", p=128),
        )
        nc.vector.reciprocal(out=rsp[:, 1, :, :], in_=rsp[:, 0, :, :])
        nc.gpsimd.dma_start(
            out=rr_scr[he : he + 2, :].rearrange("h (p q) -> p h q", p=128),
            in_=rsp[:, 1, :, :],
        )
        for h, dma_eng in ((he, nc.sync), (ho, nc.scalar)):
            o_f = of_live.pop(h)
            rb = rb_pool.tile([64, 512], f32, tag="rb")
            src = rr_scr[h : h + 1, :]
            bcast = bass.AP(
                tensor=src.tensor, offset=src.offset, ap=[[0, 64]] + src.ap[1:]
            )
            dma_eng.dma_start(out=rb[:, :], in_=bcast)
            if h % 2 == 0:
                nc.vector.tensor_mul(
                    out=aoT[jt][0:64, :], in0=o_f[0:64, :], in1=rb[:, :]
                )
            else:
                ao_s = ao_pool.tile([64, 512], bf16, tag="aos")
                nc.vector.tensor_mul(out=ao_s[:, :], in0=o_f[0:64, :], in1=rb[:, :])
                nc.scalar.dma_start(out=aoT[jt][64:128, :], in_=ao_s[:, :])

    def outproj(st, mb):
        ps = pj.tile([128, 512], f32, tag="pp")
        for t in range(JT):
            nc.tensor.matmul(
                ps[:, :],
                lhsT=aoT[t][:, st * 128 : (st + 1) * 128],
                rhs=wo_sb[:, t, mb * 512 : (mb + 1) * 512],
                start=(t == 0),
                stop=(t == JT - 1),
            )
        ob = out_pool.tile([128, 512], f32, tag="ob")
        nc.vector.tensor_add(
            out=ob[:, :],
            in0=ps[:, :],
            in1=bo_bc[:, mb * 512 : (mb + 1) * 512],
        )
        nc.sync.dma_start(
            out=out[st * 128 : (st + 1) * 128, mb * 512 : (mb + 1) * 512],
            in_=ob[:, :],
        )

    # ---- emission: interleave proj / scores+exp / pv so the PE always has
    # dense matmul work while ScalarE runs the exps.
    # v (hb=0) first: its loads arrive first so nothing FIFO-blocks, and
    # PV can start as soon as the first pair's exps land.
    for skt in range(SKT):
        vproj(skt, 0)
    kqproj(wk_sb, xkT, bk_col, kT, 0, 2)
    kqproj(wq_sb, xqT, bq_col, qT, 0, 1)
    for jt in range(JT):
        scores_pair(jt)
        if jt + 1 < JT:
            kqproj(wk_sb, xkT, bk_col, kT, jt + 1, 2)
            kqproj(wq_sb, xqT, bq_col, qT, jt + 1, 1)
        if jt < 4:
            vproj(2 * jt, 1)
            vproj(2 * jt + 1, 1)
        if jt >= 1:
            pv_head(2 * (jt - 1))
            pv_head(2 * (jt - 1) + 1)
            pair_norm(jt - 1)
    for h in range(2 * (JT - 1), H):
        pv_head(h)
    pair_norm(JT - 1)
    for st in range(STQ):
        for mb in range(2):
            outproj(st, mb)


def _build():
    import concourse.tile as tile
    from concourse import bacc

    from contextlib import ExitStack

    nc = bacc.Bacc(
        "TRN2", target_bir_lowering=False, debug=False, num_devices=NCORES
    )
    with tile.TileContext(nc) as tc:
        with ExitStack() as ctx:
            _emit(tc, ctx)
    nc.compile()
    return nc


def _get_nc():
    if "nc" not in _CACHED:
        _CACHED["nc"] = _build()
    return _CACHED["nc"]


def _build_in_maps(inputs):
    import ml_dtypes

    bf16 = ml_dtypes.bfloat16

    def cvt(a):
        return np.asarray(a, dtype=np.float32).astype(bf16)

    def pack(a2d):
        # [R, M] -> [128, R//128, M] partition-major contiguous
        r, m = a2d.shape
        return np.ascontiguousarray(
            a2d.reshape(r // 128, 128, m).transpose(1, 0, 2)
        )

    queries = np.asarray(inputs["queries"], dtype=np.float32)
    keys = np.asarray(inputs["keys"], dtype=np.float32)
    values = np.asarray(inputs["values"], dtype=np.float32)
    shared = {
        "wq": pack(cvt(inputs["Wq"])),
        "wk": pack(cvt(inputs["Wk"])),
        "wv": np.ascontiguousarray(
            cvt(inputs["Wv"]).reshape(CT, 128, 2, 512).transpose(1, 2, 0, 3)
        ),
        "wo": pack(cvt(inputs["Wo"])),
        "bq": np.ascontiguousarray(np.asarray(inputs["bq"], np.float32)),
        "bk": np.ascontiguousarray(np.asarray(inputs["bk"], np.float32)),
        "bv": np.ascontiguousarray(np.asarray(inputs["bv"], np.float32)),
        "bo": np.ascontiguousarray(np.asarray(inputs["bo"], np.float32)),
    }
    in_maps = []
    for c in range(NCORES):
        b, hh = c // 2, c % 2
        in_maps.append(
            {
                "xqt": pack(cvt(queries[b, hh * SQ : (hh + 1) * SQ]).T),
                "xkt": pack(cvt(keys[b]).T),
                "xvt": np.ascontiguousarray(
                    cvt(values[b]).T.reshape(CT, 128, SKT, 128).transpose(1, 2, 0, 3)
                ),
                **shared,
            }
        )
    return in_maps


def kernel(**inputs):
    from concourse.bass_utils import run_bass_kernel_spmd

    nc = _get_nc()
    in_maps = _build_in_maps(inputs)
    res = run_bass_kernel_spmd(nc, in_maps, list(range(NCORES)))
    full = np.empty((B, S, C), dtype=np.float32)
    for c in range(NCORES):
        b, hh = c // 2, c % 2
        full[b, hh * SQ : (hh + 1) * SQ] = res.results[c]["out"]
    return full


# revision 39
# speedup vs baseline: 1.0496x; 1.0496x over previous
"""Trainium2 Bass kernel for multi-head attention (B=4, S=1024, D=1024, H=16).

Sharding: 8 cores = batch(4) x query-half(2). Each core computes the full
attention output for its 512 query rows of its batch (all 16 heads), so the
per-core outputs are disjoint slices of the final [4, 1024, 1024] output and
the gather is a pure concatenation -- no cross-core communication.

Host-side prep (part of the sharding step): inputs are cast to bf16 and
pre-transposed so every DRAM tensor is already in the layout the matmuls
consume (x^T with d_model on partitions, weights in natural [c, hd] layout).
This removes all on-device casts/transposes and halves HBM load traffic.

Per-core dataflow (all matmuls bf16, fp32 PSUM accumulation):
  qT = Wq^T xqT + bq   kT = Wk^T xkT + bk   (bias fused into the PSUM
    drain as a DVE per-partition scalar add)
  v[sk, h, d|1] = xvT^T Wv + bv             (ones column -> rowsum; bias
    fused into the drain via a partition-replicated bias tile)
  S^T[h] per head pair: even head on PE rows 0-63, odd on 64-127
    (row-packed tile_position -> the two streams run concurrently)
  P^T = exp(S^T / 8)          (ScalarE, PSUM -> bf16 SBUF)
  [out^T[h]; rowsum] = [v_h | 1]^T P^T
  attn^T = out^T * (1/rowsum) (wide [128,8] reciprocal via DRAM bounce +
    stride-0 partition-broadcast DMA)
  o = attn^T^T Wo + bo

Scheduling: loads are dependency-chained in consumption order across the
two HWDGE rings (x on sync, weights on scalar) so the first-needed tensors
get full HBM bandwidth; emission interleaves v/k/q projections with
attention head pairs so the tensor engine always has dense work while
ScalarE chews through the exps (keeps the PE HAM clock-gate warm); per-jt
output tiles keep consumer dependencies exact so the output projection
overlaps the last heads' normalize chains.
"""

import sys

if "/opt/trn_rl_repo" not in sys.path:
    sys.path.insert(0, "/opt/trn_rl_repo")

import numpy as np

B = 4
S = 1024
C = 1024          # d_model
H = 16            # heads
D = 64            # head dim
HD = H * D        # 1024
SQ = S // 2       # queries per core
NCORES = 8
SCALE = 0.125     # 1/sqrt(D)

CT = C // 128     # 8 contraction tiles
JT = HD // 128    # 8 head-pair tiles
SKT = S // 128    # 8 key tiles
STQ = SQ // 128   # 4 query row-tiles

_CACHED = {}


def _emit(tc, ctx):
    import concourse.bass as bass
    from concourse import mybir

    nc = tc.nc
    f32 = mybir.dt.float32
    bf16 = mybir.dt.bfloat16
    Exp = mybir.ActivationFunctionType.Exp

    # ---- DRAM I/O (bf16, pre-transposed AND partition-major packed on
    # host: [128, ct, m] so every partition line is one contiguous run) ----
    xqt = nc.dram_tensor("xqt", [128, CT, SQ], bf16, kind="ExternalInput").ap()
    xkt = nc.dram_tensor("xkt", [128, CT, S], bf16, kind="ExternalInput").ap()
    xvt = nc.dram_tensor("xvt", [128, SKT, CT, 128], bf16, kind="ExternalInput").ap()
    wq = nc.dram_tensor("wq", [128, CT, HD], bf16, kind="ExternalInput").ap()
    wk = nc.dram_tensor("wk", [128, CT, HD], bf16, kind="ExternalInput").ap()
    wv = nc.dram_tensor("wv", [128, 2, CT, 512], bf16, kind="ExternalInput").ap()
    wo = nc.dram_tensor("wo", [128, JT, C], bf16, kind="ExternalInput").ap()
    bq = nc.dram_tensor("bq", [HD], f32, kind="ExternalInput").ap()
    bk = nc.dram_tensor("bk", [HD], f32, kind="ExternalInput").ap()
    bv = nc.dram_tensor("bv", [HD], f32, kind="ExternalInput").ap()
    bo = nc.dram_tensor("bo", [C], f32, kind="ExternalInput").ap()
    out = nc.dram_tensor("out", [SQ, C], f32, kind="ExternalOutput").ap()

    # rowsum bounce rows (reshape for wide reciprocal + partition broadcast)
    rs_scr = nc.dram_tensor("rs_scr", [H, SQ], f32).ap()
    rr_scr = nc.dram_tensor("rr_scr", [H, SQ], f32).ap()

    # ---- long-lived SBUF ----
    persist = ctx.enter_context(tc.tile_pool(name="persist", bufs=1))
    wq_sb = persist.tile([128, CT, HD], bf16)
    wk_sb = persist.tile([128, CT, HD], bf16)
    wv_sb = persist.tile([128, 2, CT, 512], bf16)
    wo_sb = persist.tile([128, JT, C], bf16)
    xqT = persist.tile([128, CT, SQ], bf16)
    xkT = persist.tile([128, CT, S], bf16)
    xvT = persist.tile([128, SKT, CT, 128], bf16)
    # per-jt / per-half tiles so consumers depend only on their producers
    qT = [persist.tile([128, SQ], bf16, name=f"qT{j}") for j in range(JT)]
    kT = [persist.tile([128, S], bf16, name=f"kT{j}") for j in range(JT)]
    v_sb = [
        persist.tile([128, SKT, H // 2, D + 1], bf16, name=f"vsb{b}")
        for b in range(2)
    ]
    aoT = [persist.tile([128, SQ], bf16, name=f"aoT{j}") for j in range(JT)]
    bq_col = persist.tile([128, JT], f32)
    bk_col = persist.tile([128, JT], f32)
    bv_bc = persist.tile([128, HD], f32)
    bo_bc = persist.tile([128, C], f32)
    ones65 = persist.tile([65, 64], f32)

    # ---- working pools ----
    pj = ctx.enter_context(tc.tile_pool(name="proj_psum", bufs=2, space="PSUM"))
    sp = ctx.enter_context(tc.tile_pool(name="st_psum", bufs=2, space="PSUM"))
    vp = ctx.enter_context(tc.tile_pool(name="pv_psum", bufs=2, space="PSUM"))
    pt_pool = ctx.enter_context(tc.tile_pool(name="pt", bufs=12))
    of_pool = ctx.enter_context(tc.tile_pool(name="of", bufs=4))
    rsp_pool = ctx.enter_context(tc.tile_pool(name="rsp", bufs=2))
    rb_pool = ctx.enter_context(tc.tile_pool(name="rb", bufs=3))
    ao_pool = ctx.enter_context(tc.tile_pool(name="ao_stage", bufs=3))
    out_pool = ctx.enter_context(tc.tile_pool(name="out_sb", bufs=3))

    # ---- loads: nc.sync HWDGE ring is FIFO per engine, so emission order
    # is arrival order. vproj work comes first in the schedule, so xv/wv
    # lead; remaining tensors follow in consumption order, split so the
    # first consumer can start at partial load. Tiny bias rows ride the
    # gpsimd (SWDGE) ring so they don't delay the big streams.
    # biases: bq/bk as per-partition columns (fused into the PSUM-drain
    # copies), bv/bo replicated across partitions via stride-0 DMA.
    with nc.allow_non_contiguous_dma(reason="tiny transposed bias loads"):
        nc.gpsimd.dma_start(out=bq_col[:, :], in_=bq.rearrange("(t p) -> p t", p=128))
        nc.gpsimd.dma_start(out=bk_col[:, :], in_=bk.rearrange("(t p) -> p t", p=128))
    for dst, src in ((bv_bc, bv), (bo_bc, bo)):
        s = src.rearrange("(o m) -> o m", o=1)
        rep = bass.AP(tensor=s.tensor, offset=s.offset, ap=[[0, 128]] + s.ap[1:])
        nc.gpsimd.dma_start(out=dst[:, :], in_=rep)
    # x tensors on the sync HWDGE ring, weights on the scalar HWDGE ring.
    # Transfers are grouped in consumption-order pairs; each group is
    # dependency-chained on the previous so two transfers at a time split
    # the HBM bandwidth instead of all of them at once.
    from concourse.bass import _add_dep_helper

    d_xv0 = nc.sync.dma_start(out=xvT[:, 0:4, :, :], in_=xvt[:, 0:4, :, :])
    d_wv0 = nc.scalar.dma_start(out=wv_sb[:, 0, :, :], in_=wv[:, 0, :, :])
    d_xv1 = nc.sync.dma_start(out=xvT[:, 4:8, :, :], in_=xvt[:, 4:8, :, :])
    d_wv1 = nc.scalar.dma_start(out=wv_sb[:, 1, :, :], in_=wv[:, 1, :, :])
    d_xk = nc.sync.dma_start(out=xkT[:, :, :], in_=xkt[:, :, :])
    d_wk = nc.scalar.dma_start(out=wk_sb[:, :, :], in_=wk[:, :, :])
    d_xq = nc.sync.dma_start(out=xqT[:, :, :], in_=xqt[:, :, :])
    d_wq = nc.scalar.dma_start(out=wq_sb[:, :, :], in_=wq[:, :, :])
    d_wo = nc.scalar.dma_start(out=wo_sb[:, :, :], in_=wo[:, :, :])
    for later, earlier in (
        (d_xv1, d_xv0),
        (d_wv1, d_wv0),
        (d_xk, d_xv1),
        (d_wk, d_wv1),
        (d_xq, d_xk),
        (d_wq, d_wk),
        (d_wo, d_wq),
    ):
        _add_dep_helper(later.ins, earlier.ins, sync=True, reason="load order")

    nc.vector.memset(ones65[:, :], 1.0)
    nc.vector.memset(v_sb[0][:, :, :, D : D + 1], 1.0)
    nc.vector.memset(v_sb[1][:, :, :, D : D + 1], 1.0)

    def kqproj(w_sb, x_sb, b_col, o_sb, jt, nsb):
        # o^T[j, s] = W^T x^T + b  (contraction over c; bias fused into the
        # PSUM drain as a per-partition scalar add)
        for sb in range(nsb):
            ps = pj.tile([128, 512], f32, tag="pp")
            for ct in range(CT):
                nc.tensor.matmul(
                    ps[:, :],
                    lhsT=w_sb[:, ct, jt * 128 : (jt + 1) * 128],
                    rhs=x_sb[:, ct, sb * 512 : (sb + 1) * 512],
                    start=(ct == 0),
                    stop=(ct == CT - 1),
                )
            nc.vector.tensor_scalar_add(
                out=o_sb[jt][:, sb * 512 : (sb + 1) * 512],
                in0=ps[:, :],
                scalar1=b_col[:, jt : jt + 1],
            )

    def vproj(skt, hb):
        # v[sk, hd-half] = xv^T^T Wv + bv (bias fused into the PSUM drain)
        ps = pj.tile([128, 512], f32, tag="pp")
        for ct in range(CT):
            nc.tensor.matmul(
                ps[:, :],
                lhsT=xvT[:, skt, ct, :],
                rhs=wv_sb[:, hb, ct, :],
                start=(ct == 0),
                stop=(ct == CT - 1),
            )
        nc.vector.tensor_add(
            out=v_sb[hb][:, skt, :, 0:D],
            in0=ps.rearrange("p (h d) -> p h d", d=D),
            in1=bv_bc[:, hb * 512 : (hb + 1) * 512].rearrange(
                "p (h d) -> p h d", d=D
            ),
        )

    pt_live = {}

    def scores_pair(jt):
        # even head on PE rows 0-63, odd head on rows 64-127 (concurrent)
        pe, po = [], []
        for g in range(4):
            st_e = sp.tile([128, 2, 512], f32, tag="st")
            st_o = sp.tile([128, 2, 512], f32, tag="st")
            for i in range(2):
                skt = 2 * g + i
                nc.tensor.matmul(
                    st_e[:, i, :],
                    lhsT=kT[jt][0:64, skt * 128 : (skt + 1) * 128],
                    rhs=qT[jt][0:64, :],
                    start=True,
                    stop=True,
                )
                nc.tensor.matmul(
                    st_o[:, i, :],
                    lhsT=kT[jt][64:128, skt * 128 : (skt + 1) * 128],
                    rhs=qT[jt][64:128, :],
                    start=True,
                    stop=True,
                )
            p_e = pt_pool.tile([128, 2, 512], bf16, tag="pt")
            p_o = pt_pool.tile([128, 2, 512], bf16, tag="pt")
            nc.scalar.activation(out=p_e[:, :, :], in_=st_e[:, :, :], func=Exp, scale=SCALE)
            nc.scalar.activation(out=p_o[:, :, :], in_=st_o[:, :, :], func=Exp, scale=SCALE)
            pe.append(p_e)
            po.append(p_o)
        pt_live[2 * jt] = pe
        pt_live[2 * jt + 1] = po

    of_live = {}

    def pv_head(h):
        pts = pt_live.pop(h)
        o_ps = vp.tile([65, 512], f32, tag="pv")
        for skt in range(SKT):
            nc.tensor.matmul(
                o_ps[:, :],
                lhsT=v_sb[h // 8][:, skt, h % 8, :],
                rhs=pts[skt // 2][:, skt % 2, :],
                start=(skt == 0),
                stop=(skt == SKT - 1),
            )
        # one copy frees the PSUM bank immediately (rows 0-63 out^T, 64 sum)
        o_f = of_pool.tile([65, 512], f32, tag="of")
        nc.vector.tensor_copy(out=o_f[:, :], in_=o_ps[:, :])
        nc.gpsimd.dma_start(out=rs_scr[h : h + 1, :], in_=o_f[64:65, :])
        of_live[h] = o_f

    def pair_norm(jt):
        he, ho = 2 * jt, 2 * jt + 1
        if jt == JT - 1:
            # last pair: its chain is exposed at the tail, so trade a slow
            # [1,512] DVE reciprocal for a DRAM-free path (broadcast via a
            # K=1 matmul into a spare score-psum slot).
            for h in (he, ho):
                o_f = of_live.pop(h)
                rsr = of_pool.tile([65, 512], f32, tag="of")
                nc.vector.reciprocal(out=rsr[64:65, :], in_=o_f[64:65, :])
                rbp = sp.tile([128, 2, 512], f32, tag="st")
                nc.tensor.matmul(
                    rbp[0:64, 0, :],
                    lhsT=ones65[64:65, 0:64],
                    rhs=rsr[64:65, :],
                    start=True,
                    stop=True,
                )
                if h % 2 == 0:
                    nc.vector.tensor_mul(
                        out=aoT[jt][0:64, :], in0=o_f[0:64, :], in1=rbp[0:64, 0, :]
                    )
                else:
                    ao_s = ao_pool.tile([64, 512], bf16, tag="aos")
                    nc.vector.tensor_mul(
                        out=ao_s[:, :], in0=o_f[0:64, :], in1=rbp[0:64, 0, :]
                    )
                    nc.scalar.dma_start(out=aoT[jt][64:128, :], in_=ao_s[:, :])
            return
        # reciprocal done wide ([128,8] for the head pair via DRAM bounce:
        # DVE recip cost is per-lane free-size, so [1,512] is ~9x slower).
        rsp = rsp_pool.tile([128, 2, 2, 4], f32, tag="rsp")
        nc.gpsimd.dma_start(
            out=rsp[:, 0, :, :],
            in_=rs_scr[he : he + 2, :].rearrange("h (p q) -> p h q", p=128),
        )
        nc.vector.reciprocal(out=rsp[:, 1, :, :], in_=rsp[:, 0, :, :])
        nc.gpsimd.dma_start(
            out=rr_scr[he : he + 2, :].rearrange("h (p q) -> p h q", p=128),
            in_=rsp[:, 1, :, :],
        )
        for h, dma_eng in ((he, nc.sync), (ho, nc.scalar)):
            o_f = of_live.pop(h)
            rb = rb_pool.tile([64, 512], f32, tag="rb")
            src = rr_scr[h : h + 1, :]
            bcast = bass.AP(
                tensor=src.tensor, offset=src.offset, ap=[[0, 64]] + src.ap[1:]
            )
            dma_eng.dma_start(out=rb[:, :], in_=bcast)
            if h % 2 == 0:
                nc.vector.tensor_mul(
                    out=aoT[jt][0:64, :], in0=o_f[0:64, :], in1=rb[:, :]
                )
            else:
                ao_s = ao_pool.tile([64, 512], bf16, tag="aos")
                nc.vector.tensor_mul(out=ao_s[:, :], in0=o_f[0:64, :], in1=rb[:, :])
                nc.scalar.dma_start(out=aoT[jt][64:128, :], in_=ao_s[:, :])

    def outproj(st, mb):
        ps = pj.tile([128, 512], f32, tag="pp")
        for t in range(JT):
            nc.tensor.matmul(
                ps[:, :],
                lhsT=aoT[t][:, st * 128 : (st + 1) * 128],
                rhs=wo_sb[:, t, mb * 512 : (mb + 1) * 512],
                start=(t == 0),
                stop=(t == JT - 1),
            )
        ob = out_pool.tile([128, 512], f32, tag="ob")
        nc.vector.tensor_add(
            out=ob[:, :],
            in0=ps[:, :],
            in1=bo_bc[:, mb * 512 : (mb + 1) * 512],
        )
        nc.sync.dma_start(
            out=out[st * 128 : (st + 1) * 128, mb * 512 : (mb + 1) * 512],
            in_=ob[:, :],
        )

    # ---- emission: interleave proj / scores+exp / pv so the PE always has
    # dense matmul work while ScalarE runs the exps.
    # v (hb=0) first: its loads arrive first so nothing FIFO-blocks, and
    # PV can start as soon as the first pair's exps land.
    for hb in range(2):
        for skt in range(SKT):
            vproj(skt, hb)
    kqproj(wk_sb, xkT, bk_col, kT, 0, 2)
    kqproj(wq_sb, xqT, bq_col, qT, 0, 1)
    for jt in range(JT):
        scores_pair(jt)
        if jt + 1 < JT:
            kqproj(wk_sb, xkT, bk_col, kT, jt + 1, 2)
            kqproj(wq_sb, xqT, bq_col, qT, jt + 1, 1)
        if jt >= 1:
            pv_head(2 * (jt - 1))
            pv_head(2 * (jt - 1) + 1)
            pair_norm(jt - 1)
    for h in range(2 * (JT - 1), H):
        pv_head(h)
    pair_norm(JT - 1)
    for st in range(STQ):
        for mb in range(2):
            outproj(st, mb)


def _build():
    import concourse.tile as tile
    from concourse import bacc

    from contextlib import ExitStack

    nc = bacc.Bacc(
        "TRN2", target_bir_lowering=False, debug=False, num_devices=NCORES
    )
    with tile.TileContext(nc) as tc:
        with ExitStack() as ctx:
            _emit(tc, ctx)
    nc.compile()
    return nc


def _get_nc():
    if "nc" not in _CACHED:
        _CACHED["nc"] = _build()
    return _CACHED["nc"]


def _build_in_maps(inputs):
    import ml_dtypes

    bf16 = ml_dtypes.bfloat16

    def cvt(a):
        return np.asarray(a, dtype=np.float32).astype(bf16)

    def pack(a2d):
        # [R, M] -> [128, R//128, M] partition-major contiguous
        r, m = a2d.shape
        return np.ascontiguousarray(
            a2d.reshape(r // 128, 128, m).transpose(1, 0, 2)
        )

    queries = np.asarray(inputs["queries"], dtype=np.float32)
    keys = np.asarray(inputs["keys"], dtype=np.float32)
    values = np.asarray(inputs["values"], dtype=np.float32)
    shared = {
        "wq": pack(cvt(inputs["Wq"])),
        "wk": pack(cvt(inputs["Wk"])),
        "wv": np.ascontiguousarray(
            cvt(inputs["Wv"]).reshape(CT, 128, 2, 512).transpose(1, 2, 0, 3)
        ),
        "wo": pack(cvt(inputs["Wo"])),
        "bq": np.ascontiguousarray(np.asarray(inputs["bq"], np.float32)),
        "bk": np.ascontiguousarray(np.asarray(inputs["bk"], np.float32)),
        "bv": np.ascontiguousarray(np.asarray(inputs["bv"], np.float32)),
        "bo": np.ascontiguousarray(np.asarray(inputs["bo"], np.float32)),
    }
    in_maps = []
    for c in range(NCORES):
        b, hh = c // 2, c % 2
        in_maps.append(
            {
                "xqt": pack(cvt(queries[b, hh * SQ : (hh + 1) * SQ]).T),
                "xkt": pack(cvt(keys[b]).T),
                "xvt": np.ascontiguousarray(
                    cvt(values[b]).T.reshape(CT, 128, SKT, 128).transpose(1, 2, 0, 3)
                ),
                **shared,
            }
        )
    return in_maps


def kernel(**inputs):
    from concourse.bass_utils import run_bass_kernel_spmd

    nc = _get_nc()
    in_maps = _build_in_maps(inputs)
    res = run_bass_kernel_spmd(nc, in_maps, list(range(NCORES)))
    full = np.empty((B, S, C), dtype=np.float32)
    for c in range(NCORES):
        b, hh = c // 2, c % 2
        full[b, hh * SQ : (hh + 1) * SQ] = res.results[c]["out"]
    return full
